# revision 24
# baseline (speedup 1.0000x reference)
"""DeepSpeedMLP (pre-LN fp32 path) on 8 Trainium2 NeuronCores.

Sharding: data-parallel over tokens (8192 tokens -> 1024/core).  Each core
streams the full inter_w (bf16) / output_w (fp8e4, host-scaled by W2S)
exactly once per 512-token chunk while holding the transposed LN
activations, the gelu activations of the current I-block, and the output
accumulator SBUF-resident.  GEMM1 runs in bf16; GEMM2 runs in fp8e4 with
perf_mode=DoubleRow (2 fp8 MACs/cell/cycle) with fp32 PSUM accumulation;
the residual path stays fp32 end to end.

Per-core pipeline (2 chunks of 512 tokens), software-pipelined across
chunks — stage 1 of chunk c+1 is emitted in the middle of chunk c's GEMM
stream so its DVE/ACT/DMA work hides under PE time:
  stage 1: res = input+(residual+bias); LN; 128x128 PE transposes -> lnT
           (bf16, gamma/beta fused into the PSUM->SBUF copy); res is stored
           to out (the final residual-add source).
  stage 2: for each 2048-wide I-block:
           GEMM1  psum[i,t] += w1[k,i].T @ lnT[k,t]  (weights stationary)
           gelu(psum + b1) -> inter (fp8e4, [i,t] layout, no transpose)
           GEMM2  psum[h,t] += w2[2i+ko,h].T @ inter[2i+ko,t] (DoubleRow)
           out_sb[h,t] += psum  (DVE, bf16 accumulator; b2*W2S folded
           into the block-0 eviction as a per-partition vector add)
  stage 3: PE-transpose out_sb back to [t,h], descale by 1/W2S on ACT,
           add the res tile loaded from out, store.
"""
import sys
if '/opt/trn_rl_repo' not in sys.path:
    sys.path.insert(0, '/opt/trn_rl_repo')

import numpy as np
import ml_dtypes
import concourse.bass as bass
import concourse.mybir as mybir
import concourse.tile as tile
from concourse import bacc
from concourse.bass_utils import run_bass_kernel_spmd

dt = mybir.dt
AF = mybir.ActivationFunctionType
ALU = mybir.AluOpType
PM = mybir.MatmulPerfMode

N_CORES = 8
B, S, HIDDEN, INTER = 4, 2048, 4096, 16384
TOK = B * S
T = TOK // N_CORES       # tokens per core (1024)
TCH = 512                # token-chunk width
IBLK = 2048              # I-block width
IW = 128                 # w1 stream-tile width (i)
HW = 512                 # w2 stream-tile width (h)
EPS = 1e-5
BF = dt.bfloat16
F8 = dt.float8e4
W2S = 128.0              # host-side scale on w2/b2 so fp8 values are ~N(0,1)


def _build_nc(H, I, T, repeat=1, act=AF.Gelu_apprx_tanh):
    KS = H // 128        # 32 h-slabs
    NIB = I // IW        # w1 stream tiles
    NB = I // IBLK       # 8 I-blocks
    ISB = IBLK // 128    # 16 i-slabs per block
    HB = H // HW         # 8 w2 h-groups
    NC_ = T // TCH       # 2 token chunks
    NTT = TCH // 128     # 4 token tiles per chunk
    SW = 512             # stage-1 strip width
    NS = H // SW

    nc = bacc.Bacc(None, target_bir_lowering=False)
    P = nc.declare_dram_parameter
    x_d = P("x", [T, H], dt.float32, isOutput=False)
    r_d = P("r", [T, H], dt.float32, isOutput=False)
    g_d = P("gamma_t", [128, KS], dt.float32, isOutput=False)
    be_d = P("beta_t", [128, KS], dt.float32, isOutput=False)
    w1_d = P("w1q", [128, NIB, KS, IW], BF, isOutput=False)
    b1_d = P("b1_t", [128, I // 128], dt.float32, isOutput=False)
    # w2 is fp8e4 (scaled by W2S host-side), laid out in DoubleRow ko-pairs:
    # w2q[p, hb, i2, ko, hh] = W2S * w2[(2*i2+ko)*128 + p, hb*HW + hh]
    w2_d = P("w2q", [128, HB, I // 256, 2, HW], F8, isOutput=False)
    # b2*W2S transposed like gamma: b2t[p, hs] = W2S * b2[hs*128 + p]
    b2_d = P("b2_t", [128, KS], dt.float32, isOutput=False)
    id_d = P("ident", [128, 128], dt.float32, isOutput=False)
    o_d = P("out", [T, H], dt.float32, isOutput=True)

    with tile.TileContext(nc) as tc:
        with (
            tc.tile_pool(name="const", bufs=1) as constp,
            tc.tile_pool(name="psum", bufs=6, space="PSUM") as psum,
            tc.tile_pool(name="psT", bufs=2, space="PSUM") as psT,
            tc.tile_pool(name="lnT", bufs=2) as lnTp,
            tc.tile_pool(name="osb", bufs=1) as osbp,
            tc.tile_pool(name="s1in", bufs=2) as inp,
            tc.tile_pool(name="s1res", bufs=2) as resp,
            tc.tile_pool(name="s1st", bufs=2 * NTT) as stp,
            tc.tile_pool(name="w1s", bufs=3) as w1p,
            tc.tile_pool(name="w2s", bufs=2) as w2p,
            tc.tile_pool(name="int", bufs=2) as intp,
            tc.tile_pool(name="fin", bufs=2) as finp,
        ):
            ident = constp.tile([128, 128], dt.float32)
            nc.sync.dma_start(out=ident[:], in_=id_d[:])
            identb = constp.tile([128, 128], BF)
            nc.vector.tensor_copy(identb[:], ident[:])
            g_sb = constp.tile([128, KS], dt.float32)
            nc.sync.dma_start(out=g_sb[:], in_=g_d[:])
            be_sb = constp.tile([128, KS], dt.float32)
            nc.sync.dma_start(out=be_sb[:], in_=be_d[:])
            b1_sb = constp.tile([128, I // 128], dt.float32)
            nc.sync.dma_start(out=b1_sb[:], in_=b1_d[:])
            b2_sb = constp.tile([128, KS], dt.float32)
            nc.sync.dma_start(out=b2_sb[:], in_=b2_d[:])

            # PE clock ramp: the tensor engine runs at reduced clock until
            # ~4-5us of sustained use.  Burn that in on the identity tile
            # while stage-1's x/r DMAs and LN stats run, so the first real
            # matmuls start warm.
            for _ in range(32):
                wpt = psT.tile([128, 512], dt.float32, name="psT")
                nc.tensor.transpose(wpt[:, 0:128], ident[:], ident[:])

            def emit_stage1(c):
                """residual add + LN + PE transpose -> lnT tile (bf16)."""
                lnT = lnTp.tile([128, KS, TCH], BF, name="lnT")
                for tt in range(NTT):
                    tr = slice(c * TCH + tt * 128, c * TCH + (tt + 1) * 128)
                    res = resp.tile([128, H], dt.float32, name="res")
                    s1 = stp.tile([128, 1], dt.float32, name="s1")
                    s2 = stp.tile([128, 1], dt.float32, name="s2")
                    for s in range(NS):
                        cs = slice(s * SW, (s + 1) * SW)
                        xs = inp.tile([128, SW], dt.float32, name="xt")
                        rs = inp.tile([128, SW], dt.float32, name="rt")
                        nc.sync.dma_start(out=xs[:], in_=x_d[tr, cs])
                        nc.sync.dma_start(out=rs[:], in_=r_d[tr, cs])
                        nc.vector.tensor_add(res[:, cs], xs[:], rs[:])
                    nc.vector.tensor_reduce(s1[:], res[:],
                                            mybir.AxisListType.X, ALU.add)
                    for s in range(NS):
                        cs = slice(s * SW, (s + 1) * SW)
                        junk = inp.tile([128, SW], dt.float32, name="xt")
                        s2p = stp.tile([128, 1], dt.float32, name="s2p")
                        nc.scalar.activation(junk[:], res[:, cs], AF.Square,
                                             accum_out=s2p[:])
                        if s == 0:
                            nc.vector.tensor_copy(s2[:], s2p[:])
                        else:
                            nc.vector.tensor_add(s2[:], s2[:], s2p[:])
                    mu = stp.tile([128, 1], dt.float32, name="mu")
                    nc.vector.tensor_scalar_mul(mu[:], s1[:], 1.0 / H)
                    mu2 = stp.tile([128, 1], dt.float32, name="mu2")
                    nc.vector.tensor_mul(mu2[:], mu[:], mu[:])
                    var = stp.tile([128, 1], dt.float32, name="var")
                    nc.vector.tensor_scalar(var[:], s2[:], 1.0 / H,
                                            float(EPS), ALU.mult, ALU.add)
                    nc.vector.tensor_sub(var[:], var[:], mu2[:])
                    sd = stp.tile([128, 1], dt.float32, name="sd")
                    nc.scalar.activation(sd[:], var[:], AF.Sqrt)
                    rstd = stp.tile([128, 1], dt.float32, name="rstd")
                    nc.vector.reciprocal(rstd[:], sd[:])
                    nmr = stp.tile([128, 1], dt.float32, name="nmr")
                    nc.vector.tensor_mul(nmr[:], mu[:], rstd[:])
                    nc.vector.tensor_scalar_mul(nmr[:], nmr[:], -1.0)
                    # res seed-store: read back by the inline stage 3 at the
                    # end of this chunk's GEMMs.
                    nc.sync.dma_start(out=o_d[tr, :], in_=res[:])
                    for s in range(NS):
                        cs = slice(s * SW, (s + 1) * SW)
                        lnp = inp.tile([128, SW], dt.float32, name="rt")
                        nc.scalar.activation(lnp[:], res[:, cs], AF.Identity,
                                             bias=nmr[:], scale=rstd[:])
                        for q in range(SW // 512):
                            pt = psT.tile([128, 512], dt.float32, name="psT")
                            for j in range(4):
                                nc.tensor.transpose(
                                    pt[:, j * 128:(j + 1) * 128],
                                    lnp[:, q * 512 + j * 128:
                                        q * 512 + (j + 1) * 128],
                                    ident[:])
                            for j in range(4):
                                k = (s * SW + q * 512) // 128 + j
                                nc.vector.tensor_scalar(
                                    lnT[:, k, tt * 128:(tt + 1) * 128],
                                    pt[:, j * 128:(j + 1) * 128],
                                    g_sb[:, k:k + 1], be_sb[:, k:k + 1],
                                    ALU.mult, ALU.add)
                return lnT

            # `repeat` re-runs the whole token loop; the work is idempotent
            # (same inputs -> same outputs), so repeat>1 builds a timing
            # variant whose device time scales linearly for slope-based
            # measurement in test.py.
            chunks = [c for _ in range(repeat) for c in range(NC_)]
            lnT_next = emit_stage1(chunks[0])
            for ci, c in enumerate(chunks):
                lnT = lnT_next
                out_sb = osbp.tile([128, KS, TCH], BF, name="osb")

                # ---- stage 2: I-blocks, GEMM1 -> gelu -> GEMM2 ----
                for b in range(NB):
                    inter = intp.tile([128, ISB, TCH], F8, name="inter")
                    for wi in range(IBLK // IW):
                        w1t = w1p.tile([128, KS, IW], BF, name="w1t")
                        nc.sync.dma_start(
                            out=w1t[:], in_=w1_d[:, b * (IBLK // IW) + wi])
                        for io in range(IW // 128):
                            pt = psum.tile([128, TCH], dt.float32, name="ps")
                            for k in range(KS):
                                nc.tensor.matmul(
                                    pt[:],
                                    w1t[:, k, io * 128:(io + 1) * 128],
                                    lnT[:, k, :],
                                    start=(k == 0), stop=(k == KS - 1))
                            islab = wi * (IW // 128) + io
                            nc.scalar.activation(
                                inter[:, islab, :], pt[:], act,
                                bias=b1_sb[:, b * ISB + islab:
                                           b * ISB + islab + 1])
                    last = (b == NB - 1)
                    for hb in range(HB):
                        w2t = w2p.tile([128, ISB // 2, 2, HW], F8, name="w2t")
                        nc.sync.dma_start(
                            out=w2t[:],
                            in_=w2_d[:, hb, b * (ISB // 2):
                                     (b + 1) * (ISB // 2)])
                        for hq in range(HW // 128):
                            hs = hb * (HW // 128) + hq
                            pt = psum.tile([128, TCH], dt.float32, name="ps")
                            for i in range(ISB // 2):
                                nc.tensor.matmul(
                                    pt[:],
                                    w2t[:, i, :, hq * 128:(hq + 1) * 128],
                                    inter[:, 2 * i:2 * i + 2, :],
                                    start=(i == 0),
                                    stop=(i == ISB // 2 - 1),
                                    perf_mode=PM.DoubleRow)
                            if b == 0:
                                # fold b2*W2S into the first eviction
                                nc.vector.tensor_scalar_add(
                                    out_sb[:, hs, :], pt[:],
                                    b2_sb[:, hs:hs + 1])
                            else:
                                nc.vector.tensor_add(
                                    out_sb[:, hs, :], out_sb[:, hs, :], pt[:])
                            # stage 3, inlined: once the 4 h-slabs of this
                            # 512-wide h-group are final, transpose back to
                            # [t,h], descale by 1/W2S (ACT), add the res tile
                            # seeded in o_d, and store — overlapping the
                            # remaining GEMM2 matmuls of the last block.
                            if last and hq == HW // 128 - 1:
                                for tt in range(NTT):
                                    tr = slice(c * TCH + tt * 128,
                                               c * TCH + (tt + 1) * 128)
                                    cs = slice(hb * HW, (hb + 1) * HW)
                                    ost = finp.tile([128, HW], dt.float32,
                                                    name="ost")
                                    nc.sync.dma_start(out=ost[:],
                                                      in_=o_d[tr, cs])
                                    pt2 = psT.tile([128, HW], BF, name="psT")
                                    for j in range(4):
                                        nc.tensor.transpose(
                                            pt2[:, j * 128:(j + 1) * 128],
                                            out_sb[:, hb * 4 + j,
                                                   tt * 128:(tt + 1) * 128],
                                            identb[:])
                                    dsc = finp.tile([128, HW], BF,
                                                    name="dsc")
                                    nc.scalar.activation(
                                        dsc[:], pt2[:], AF.Identity,
                                        scale=1.0 / W2S)
                                    nc.vector.tensor_add(ost[:], ost[:],
                                                         dsc[:])
                                    nc.sync.dma_start(out=o_d[tr, cs],
                                                      in_=ost[:])
                    # emit the next chunk's stage 1 under this chunk's GEMM
                    # stream: by block 2 the PE is ~800 us from needing the
                    # next lnT, which comfortably hides the DMA + DVE/ACT
                    # LN chain without crowding the next block's weight DMA.
                    if b == 2 and ci + 1 < len(chunks):
                        lnT_next = emit_stage1(chunks[ci + 1])
    nc.compile()
    return nc


_NC_CACHE = None


def _get_nc():
    global _NC_CACHE
    if _NC_CACHE is None:
        _NC_CACHE = _build_nc(HIDDEN, INTER, T)
    return _NC_CACHE


def build_maps(inputs):
    """Per-core input maps from the full-problem input dict."""
    H, I = HIDDEN, INTER
    KS = H // 128
    bf16 = ml_dtypes.bfloat16

    x = np.ascontiguousarray(
        np.asarray(inputs['input'], np.float32).reshape(TOK, H))
    r2 = np.asarray(inputs['residual'], np.float32).reshape(TOK, H) + \
        np.asarray(inputs['bias'], np.float32)[None, :]
    gamma_t = np.ascontiguousarray(
        np.asarray(inputs['attn_nw'], np.float32).reshape(KS, 128).T)
    beta_t = np.ascontiguousarray(
        np.asarray(inputs['attn_nb'], np.float32).reshape(KS, 128).T)
    b1_t = np.ascontiguousarray(
        np.asarray(inputs['inter_b'], np.float32).reshape(I // 128, 128).T)
    fp8 = ml_dtypes.float8_e4m3
    b2_t = np.ascontiguousarray(
        (np.asarray(inputs['output_b'], np.float32) * W2S)
        .reshape(KS, 128).T)
    w1 = np.asarray(inputs['inter_w'], np.float32)
    w2 = np.asarray(inputs['output_w'], np.float32) * W2S
    # w1q[p, ib, ks, ii] = w1[ks*128+p, ib*IW+ii]
    w1q = np.ascontiguousarray(
        w1.reshape(KS, 128, I // IW, IW).transpose(1, 2, 0, 3).astype(bf16))
    # w2q[p, hb, i2, ko, hh] = W2S * w2[(2*i2+ko)*128+p, hb*HW+hh]
    w2q = np.ascontiguousarray(
        w2.reshape(I // 256, 2, 128, H // HW, HW).transpose(2, 3, 0, 1, 4)
        .astype(fp8))
    ident = np.eye(128, dtype=np.float32)

    maps = []
    for c in range(N_CORES):
        sl = slice(c * T, (c + 1) * T)
        maps.append({
            'x': x[sl], 'r': np.ascontiguousarray(r2[sl]),
            'gamma_t': gamma_t, 'beta_t': beta_t,
            'w1q': w1q, 'b1_t': b1_t, 'w2q': w2q, 'b2_t': b2_t,
            'ident': ident,
        })
    return maps


def kernel(input, residual, residual_norm, bias, attn_nw, attn_nb,
           inter_w, inter_b, output_w, output_b, **kwargs):
    nc = _get_nc()
    maps = build_maps({
        'input': input, 'residual': residual, 'bias': bias,
        'attn_nw': attn_nw, 'attn_nb': attn_nb, 'inter_b': inter_b,
        'output_b': output_b, 'inter_w': inter_w, 'output_w': output_w,
    })
    res = run_bass_kernel_spmd(nc, maps, list(range(N_CORES)))
    out = np.concatenate([res.results[c]['out'] for c in range(N_CORES)], 0)
    return out.reshape(B, S, HIDDEN).astype(np.float32)
